# revision 5
# baseline (speedup 1.0000x reference)
"""Trainium2 Bass kernel for nn_ArthTextToDenseBlock (8-core data parallel).

Algorithm (mathematically exact reformulation of the reference scan):
  The per-step MLP gate decisions depend only on the token (and the binary
  carry fds for p_denseop, handled by evaluating both branches). So:

  Phase A (batched over all B*S tokens): six MLPs -> packed 32 head values
    per token (argmax decision *differences*, densepred logits, op logits),
    token-major [b, s, 32], staged to DRAM.
  Phase B (per-core, [128b x 512s] arrays): decisions via compares; the
    sequential carries (pointer cumsum, fds sticky bit, fpm decimal scale,
    run value v, run written-flag u) are affine/max recurrences evaluated
    with the hardware tensor_tensor_scan; final placement of per-run values
    into output positions is a per-partition local_scatter (fp32 as two
    16-bit half scatters).

  Output row = [trans_dense(512) | trans_valid(512) | trans_op(512*7)].
"""
import sys
import os
import numpy as np

try:
    import concourse.bass as bass
except ImportError:
    sys.path.insert(0, "/opt/trn_rl_repo")
    import concourse.bass as bass
import concourse.tile as tile
from concourse import mybir, bacc, library_config
from concourse.bass_utils import run_bass_kernel_spmd

AF = mybir.ActivationFunctionType
OP = mybir.AluOpType
F32 = mybir.dt.float32
BF16 = mybir.dt.bfloat16
U16 = mybir.dt.uint16
I16 = mybir.dt.int16

N_CORES = 8
B, S, D = 1024, 512, 512
BL = B // N_CORES          # 128 batch rows per core
DH = 512                   # hidden 1
DH2 = 256                  # hidden 2
EPS = 1e-5
N_STILES = int(os.environ.get("ARTH_NTILES", S // 4))  # tiles of 4 s-slices

# MLP order in phase A; head-variant order for the packed W3 matmuls
MLPS = ["p_valid", "p_move", "p_op", "p_dec", "p_densepred", "p_denseop"]
NHEADV = 7  # valid, move, op, dec, densepred, den0, den1

# packed head columns
C_DV = 0            # valid z1-z0
C_DM20, C_DM21 = 1, 2
C_DD = 3            # dec z1-z0
C_DEN0 = 4          # 5 cols: e10,e12,e13,e20,e23
C_DEN1 = 9          # 5 cols
C_DP = 14           # 10 cols densepred logits
C_OP = 24           # 7 cols op logits
NHC = 32

DEBUG = os.environ.get("ARTH_KERNEL_DEBUG", "0") == "1"

_graph_cache = {}


class Off:
    """Column offsets into the phase-A weight blob [128, WA]."""
    def __init__(self):
        self.w1 = 0                                  # 6*16 chunks * 128
        self.w2 = self.w1 + 6 * 16 * 128             # 6*8 chunks * 128
        self.w3 = self.w2 + 6 * 8 * 128              # 14 chunks * 32
        self.b1 = self.w3 + 14 * 32                  # 28 cols
        self.b2 = self.b1 + 28                       # 12 cols
        self.b3 = self.b2 + 12                       # 32 cols (partition 0)
        self.idn = self.b3 + 32                      # 128 cols identity
        self.ones = self.idn + 128                   # 512 cols of 1.0
        self.WA = self.ones + 512

    def w1_off(self, m, kc, mc):
        return self.w1 + ((m * 4 + kc) * 4 + mc) * 128

    def w2_off(self, m, kc, mc2):
        return self.w2 + ((m * 4 + kc) * 2 + mc2) * 128

    def w3_off(self, j):
        return self.w3 + j * 32

    def b1_off(self, m, mc):   # m in 0..6 (6 = denseop fds=1 variant)
        return self.b1 + m * 4 + mc

    def b2_off(self, m, mc2):
        return self.b2 + m * 2 + mc2


OFF = Off()
WB_MASK = 0      # [128, 512] step mask (t >= start_pos)
WB_ONES = 512    # [128, 512] ones
WB_W = 1024


def _pack_weights(inputs, start_pos):
    """Assemble host-side constant blobs."""
    o = OFF
    wa = np.zeros((128, o.WA), np.float32)

    def arr(t):
        return np.asarray(t, np.float32)

    w3cat = np.zeros((7 * DH2, NHC), np.float32)
    b3cat = np.zeros(NHC, np.float32)

    for m, name in enumerate(MLPS):
        w1, b1, w2, b2, w3, b3 = [arr(t) for t in inputs[name]]
        if name == "p_denseop":
            w1a, w1b = w1[:D], w1[D:]
            svec = w1b.astype(np.float64).sum(0).astype(np.float32)
            w1 = w1a
            b1_eff = [b1, (b1.astype(np.float64) + svec).astype(np.float32)]
        else:
            b1_eff = [b1]
        for kc in range(4):
            for mc in range(4):
                wa[:, o.w1_off(m, kc, mc):o.w1_off(m, kc, mc) + 128] = \
                    w1[kc * 128:(kc + 1) * 128, mc * 128:(mc + 1) * 128]
        for kc in range(4):
            for mc2 in range(2):
                wa[:, o.w2_off(m, kc, mc2):o.w2_off(m, kc, mc2) + 128] = \
                    w2[kc * 128:(kc + 1) * 128, mc2 * 128:(mc2 + 1) * 128]
        for mc in range(4):
            wa[:, o.b1_off(m, mc)] = b1_eff[0][mc * 128:(mc + 1) * 128]
        if name == "p_denseop":
            for mc in range(4):
                wa[:, o.b1_off(6, mc)] = b1_eff[1][mc * 128:(mc + 1) * 128]
        for mc2 in range(2):
            wa[:, o.b2_off(m, mc2)] = b2[mc2 * 128:(mc2 + 1) * 128]

        # W3cat block for this mlp's head variant(s)
        def blk(v):
            return slice(v * DH2, (v + 1) * DH2)
        if name == "p_valid":
            w3cat[blk(0), C_DV] = w3[:, 1] - w3[:, 0]
            b3cat[C_DV] = b3[1] - b3[0]
        elif name == "p_move":
            w3cat[blk(1), C_DM20] = w3[:, 2] - w3[:, 0]
            w3cat[blk(1), C_DM21] = w3[:, 2] - w3[:, 1]
            b3cat[C_DM20] = b3[2] - b3[0]
            b3cat[C_DM21] = b3[2] - b3[1]
        elif name == "p_op":
            w3cat[blk(2), C_OP:C_OP + 7] = w3
            b3cat[C_OP:C_OP + 7] = b3
        elif name == "p_dec":
            w3cat[blk(3), C_DD] = w3[:, 1] - w3[:, 0]
            b3cat[C_DD] = b3[1] - b3[0]
        elif name == "p_densepred":
            w3cat[blk(4), C_DP:C_DP + 10] = w3
            b3cat[C_DP:C_DP + 10] = b3
        elif name == "p_denseop":
            for v, base in ((5, C_DEN0), (6, C_DEN1)):
                w3cat[blk(v), base + 0] = w3[:, 1] - w3[:, 0]
                w3cat[blk(v), base + 1] = w3[:, 1] - w3[:, 2]
                w3cat[blk(v), base + 2] = w3[:, 1] - w3[:, 3]
                w3cat[blk(v), base + 3] = w3[:, 2] - w3[:, 0]
                w3cat[blk(v), base + 4] = w3[:, 2] - w3[:, 3]
                b3cat[base + 0] = b3[1] - b3[0]
                b3cat[base + 1] = b3[1] - b3[2]
                b3cat[base + 2] = b3[1] - b3[3]
                b3cat[base + 3] = b3[2] - b3[0]
                b3cat[base + 4] = b3[2] - b3[3]

    for j in range(14):
        wa[:, o.w3_off(j):o.w3_off(j) + NHC] = w3cat[j * 128:(j + 1) * 128, :]
    wa[0, o.b3:o.b3 + NHC] = b3cat
    wa[:, o.idn:o.idn + 128] = np.eye(128, dtype=np.float32)
    wa[:, o.ones:o.ones + 512] = 1.0

    wb = np.zeros((128, WB_W), np.float32)
    wb[:, WB_MASK + start_pos:WB_MASK + S] = 1.0
    wb[:, WB_ONES:WB_ONES + S] = 1.0
    return wa, wb


def _build_graph():
    o = OFF
    nc = bacc.Bacc("TRN2", target_bir_lowering=False, debug=False,
                   num_devices=N_CORES)
    x_d = nc.dram_tensor("x", (BL, S, D), F32, kind="ExternalInput")
    wa_d = nc.dram_tensor("wa", (128, o.WA), F32, kind="ExternalInput")
    wb_d = nc.dram_tensor("wb", (128, WB_W), F32, kind="ExternalInput")
    acc_d = nc.dram_tensor("acc", (128, S * NHC), F32, kind="ExternalOutput")
    out_d = nc.dram_tensor("out", (BL, 9 * D), F32, kind="ExternalOutput")
    if DEBUG:
        dbg_d = nc.dram_tensor("dbg", (128, 16 * S), F32, kind="ExternalOutput")

    # ---------------- Phase A ----------------
    with tile.TileContext(nc) as tc:
        nc.gpsimd.load_library(library_config.local_scatter)
        with tc.tile_pool(name="const", bufs=1) as cpool, \
             tc.tile_pool(name="xin", bufs=2) as xinp, \
             tc.tile_pool(name="xT", bufs=2) as xTp, \
             tc.tile_pool(name="h1", bufs=1) as h1p_, \
             tc.tile_pool(name="h1b", bufs=1) as h1bp, \
             tc.tile_pool(name="h2", bufs=2) as h2p_, \
             tc.tile_pool(name="hsb", bufs=2) as hsbp, \
             tc.tile_pool(name="stg", bufs=2) as stgp, \
             tc.tile_pool(name="ps1", bufs=2, space="PSUM") as ps1, \
             tc.tile_pool(name="ps2", bufs=2, space="PSUM") as ps2, \
             tc.tile_pool(name="psh", bufs=2, space="PSUM") as psh, \
             tc.tile_pool(name="pst", bufs=2, space="PSUM") as pst:

            wa = cpool.tile([128, o.WA], F32, name="wa")
            nc.sync.dma_start(wa[:], wa_d[:])
            idn = wa[:, o.idn:o.idn + 128]

            for i in range(N_STILES):
                s0 = 4 * i
                xin = xinp.tile([128, 4 * D], F32, tag="xin", name="xin")
                nc.sync.dma_start(
                    xin[:], x_d[:, s0:s0 + 4, :].rearrange("p a b -> p (a b)"))

                xT = xTp.tile([128, 4, 512], F32, tag="xT", name="xT")
                for so in range(4):
                    for kc in range(4):
                        trp = pst.tile([128, 128], F32, tag="tr", name="tr")
                        nc.tensor.transpose(
                            trp[:], xin[:, so * D + kc * 128: so * D + (kc + 1) * 128],
                            idn)
                        nc.vector.tensor_copy(
                            xT[:, kc, so * 128:(so + 1) * 128], trp[:])

                hp = psh.tile([32, 512], F32, tag="heads", name="heads")
                nc.tensor.matmul(hp[:], wa[0:1, o.b3:o.b3 + NHC],
                                 wa[0:1, o.ones:o.ones + 512],
                                 start=True, stop=False, skip_group_check=True)

                hv = 0  # head-variant counter
                for m, name in enumerate(MLPS):
                    is_den = name == "p_denseop"
                    h1 = h1p_.tile([128, 4, 512], F32, tag="h1", name="h1")
                    h1b = h1bp.tile([128, 4, 512], F32, tag="h1b", name="h1b") if is_den else None
                    for mc in range(4):
                        p1 = ps1.tile([128, 512], F32, tag="ps1", name="ps1")
                        for kc in range(4):
                            nc.tensor.matmul(
                                p1[:], wa[:, o.w1_off(m, kc, mc):o.w1_off(m, kc, mc) + 128],
                                xT[:, kc, :], start=(kc == 0), stop=(kc == 3))
                        nc.scalar.activation(
                            h1[:, mc, :], p1[:], AF.Lrelu,
                            bias=wa[:, o.b1_off(m, mc):o.b1_off(m, mc) + 1],
                            scale=1.0, alpha=0.01)
                        if is_den:
                            nc.scalar.activation(
                                h1b[:, mc, :], p1[:], AF.Lrelu,
                                bias=wa[:, o.b1_off(6, mc):o.b1_off(6, mc) + 1],
                                scale=1.0, alpha=0.01)
                    for h1var in ([h1] if not is_den else [h1, h1b]):
                        h2 = h2p_.tile([128, 2, 512], F32, tag="h2", name="h2")
                        for mc2 in range(2):
                            p2 = ps2.tile([128, 512], F32, tag="ps2", name="ps2")
                            for kc in range(4):
                                nc.tensor.matmul(
                                    p2[:], wa[:, o.w2_off(m, kc, mc2):o.w2_off(m, kc, mc2) + 128],
                                    h1var[:, kc, :], start=(kc == 0), stop=(kc == 3))
                            nc.scalar.activation(
                                h2[:, mc2, :], p2[:], AF.Lrelu,
                                bias=wa[:, o.b2_off(m, mc2):o.b2_off(m, mc2) + 1],
                                scale=1.0, alpha=0.01)
                        for kc3 in range(2):
                            j = hv * 2 + kc3
                            last = (hv == NHEADV - 1) and (kc3 == 1)
                            nc.tensor.matmul(
                                hp[:], wa[:, o.w3_off(j):o.w3_off(j) + NHC],
                                h2[:, kc3, :], start=False, stop=last,
                                skip_group_check=True)
                        hv += 1

                hs = hsbp.tile([32, 512], F32, tag="hs", name="hs")
                nc.vector.tensor_copy(hs[:], hp[:])
                stg = stgp.tile([128, 4 * NHC], F32, tag="stg", name="stg")
                for so in range(4):
                    htp = pst.tile([128, 128], F32, tag="tr", name="tr")
                    nc.tensor.transpose(
                        htp[:, :NHC], hs[:, so * 128:(so + 1) * 128],
                        idn[0:32, 0:32])
                    nc.vector.tensor_copy(
                        stg[:, so * NHC:(so + 1) * NHC], htp[:, :NHC])
                nc.sync.dma_start(acc_d[:, s0 * NHC:(s0 + 4) * NHC], stg[:])

    # ---------------- Phase B ----------------
    with tile.TileContext(nc) as tc:
        with tc.tile_pool(name="pb", bufs=1) as pb:
            acc = pb.tile([128, S * NHC], F32, tag="acc", name="acc")
            nc.sync.dma_start(acc[:], acc_d[:])
            wb = pb.tile([128, WB_W], F32, tag="wb", name="wbt")
            nc.sync.dma_start(wb[:], wb_d[:])
            mask = wb[:, WB_MASK:WB_MASK + S]
            ones = wb[:, WB_ONES:WB_ONES + S]

            a3 = acc[:].rearrange("p (s c) -> p s c", c=NHC)

            def col(j):
                return a3[:, :, j]

            def T(tag, dtype=F32, n=S):
                return pb.tile([128, n], dtype, tag=tag, name=tag)

            TT, TS, STT = nc.vector.tensor_tensor, nc.vector.tensor_scalar, \
                nc.vector.scalar_tensor_tensor

            # --- decisions ---
            nig = T("nig"); TS(nig[:], col(C_DV), 0.0, None, OP.is_le)
            t20 = T("t20"); TS(t20[:], col(C_DM20), 0.0, None, OP.is_gt)
            t21 = T("t21"); TS(t21[:], col(C_DM21), 0.0, None, OP.is_gt)
            mv2 = T("mv2"); TT(mv2[:], t20[:], t21[:], OP.logical_and)
            m_ = T("m_")
            TT(m_[:], nig[:], mv2[:], OP.logical_and)
            TT(m_[:], m_[:], mask, OP.mult)
            dsg = T("dsg")
            TS(dsg[:], col(C_DD), 0.0, None, OP.is_gt)
            TT(dsg[:], dsg[:], mask, OP.mult)

            dens = {}
            for tagb, base in (("d0", C_DEN0), ("d1", C_DEN1)):
                a1 = T("da1"); TS(a1[:], col(base + 0), 0.0, None, OP.is_gt)
                a2 = T("da2"); TS(a2[:], col(base + 1), 0.0, None, OP.is_ge)
                a3_ = T("da3"); TS(a3_[:], col(base + 2), 0.0, None, OP.is_ge)
                i1 = T("i1" + tagb)
                TT(i1[:], a1[:], a2[:], OP.logical_and)
                TT(i1[:], i1[:], a3_[:], OP.logical_and)
                b1c = T("da1"); TS(b1c[:], col(base + 3), 0.0, None, OP.is_gt)
                b2c = T("da2"); TS(b2c[:], col(base + 1), 0.0, None, OP.is_lt)
                b3c = T("da3"); TS(b3c[:], col(base + 4), 0.0, None, OP.is_ge)
                i2 = T("i2" + tagb)
                TT(i2[:], b1c[:], b2c[:], OP.logical_and)
                TT(i2[:], i2[:], b3c[:], OP.logical_and)
                dens[tagb] = (i1, i2)

            # digit = argmax index of densepred logits
            mx = T("mx")
            nc.vector.tensor_reduce(mx[:], a3[:, :, C_DP:C_DP + 10],
                                    mybir.AxisListType.X, OP.max)
            dig = T("dig"); dig2 = T("dig2")
            nc.vector.memset(dig[:], 0.0)
            cur, nxt = dig, dig2
            cj = T("cj")
            for j in range(1, 10):
                TT(cj[:], col(C_DP + j), mx[:], OP.is_ge)
                STT(nxt[:], cj[:], float(j), cur[:], OP.mult, OP.add)
                cur, nxt = nxt, cur
            dig = cur

            # fds scan -> fprev
            nm = T("nm"); TS(nm[:], m_[:], -1.0, 1.0, OP.mult, OP.add)
            d1f = T("d1f"); TT(d1f[:], nm[:], dsg[:], OP.mult)
            fsc = T("fsc")
            nc.vector.tensor_tensor_scan(fsc[:], nm[:], d1f[:], 0.0, OP.mult, OP.max)
            fp = T("fp")
            nc.vector.memset(fp[:, 0:1], 0.0)
            nc.vector.tensor_copy(fp[:, 1:S], fsc[:, 0:S - 1])

            # int/dec via fds selection
            def sel(i0, i1, tag):
                d = T("seld"); TT(d[:], i1[:], i0[:], OP.subtract)
                q = T(tag)
                TT(q[:], fp[:], d[:], OP.mult)
                TT(q[:], q[:], i0[:], OP.add)
                TT(q[:], q[:], nig[:], OP.mult)
                TT(q[:], q[:], mask, OP.mult)
                return q
            int_ = sel(dens["d0"][0], dens["d1"][0], "int_")
            dec_ = sel(dens["d0"][1], dens["d1"][1], "dec_")

            # fpm scan -> fpmp (value used within step t is the carry)
            nd = T("nd"); TS(nd[:], dec_[:], -1.0, 1.0, OP.mult, OP.add)
            t3 = T("t3"); TT(t3[:], nd[:], nm[:], OP.mult)
            af = T("af"); STT(af[:], dec_[:], 0.1, t3[:], OP.mult, OP.add)
            fpm = T("fpm")
            nc.vector.tensor_tensor_scan(fpm[:], af[:], m_[:], 1.0, OP.mult, OP.add)
            fpmp = T("fpmp")
            nc.vector.memset(fpmp[:, 0:1], 1.0)
            nc.vector.tensor_copy(fpmp[:, 1:S], fpm[:, 0:S - 1])

            # v scan
            alpha = T("alpha")
            STT(alpha[:], int_[:], 9.0, ones, OP.mult, OP.add)
            TT(alpha[:], alpha[:], nm[:], OP.mult)
            b1v = T("b1v")
            TT(b1v[:], dig[:], fpmp[:], OP.mult)
            TS(b1v[:], b1v[:], 0.1, None, OP.mult)
            TT(b1v[:], b1v[:], dec_[:], OP.mult)
            beta = T("beta")
            TT(beta[:], int_[:], dig[:], OP.mult)
            TT(beta[:], beta[:], b1v[:], OP.add)
            v = T("v")
            nc.vector.tensor_tensor_scan(v[:], alpha[:], beta[:], 0.0, OP.mult, OP.add)

            # u scan
            w_ = T("w_"); TT(w_[:], int_[:], dec_[:], OP.add)
            u = T("u")
            nc.vector.tensor_tensor_scan(u[:], nm[:], w_[:], 0.0, OP.mult, OP.max)

            # pointer cumsum
            c = T("c")
            nc.vector.tensor_tensor_scan(c[:], ones, m_[:], 0.0, OP.mult, OP.add)

            # end-of-run flags
            e = T("e")
            nc.vector.memset(e[:, S - 1:S], 1.0)
            nc.vector.tensor_copy(e[:, 0:S - 1], m_[:, 1:S])

            # scatter indices
            ok = T("ok"); TS(ok[:], c[:], float(D) - 0.5, None, OP.is_le)
            cp1 = T("cp1"); TS(cp1[:], c[:], 1.0, None, OP.add)
            mske = T("mske"); TT(mske[:], e[:], ok[:], OP.mult)
            idxe = T("idxe")
            TT(idxe[:], cp1[:], mske[:], OP.mult)
            TS(idxe[:], idxe[:], 1.0, None, OP.subtract)
            idxe16 = T("idxe16", I16)
            nc.vector.tensor_copy(idxe16[:], idxe[:])
            mskm = T("mskm"); TT(mskm[:], m_[:], ok[:], OP.mult)
            idxm = T("idxm")
            TT(idxm[:], cp1[:], mskm[:], OP.mult)
            TS(idxm[:], idxm[:], 1.0, None, OP.subtract)
            idxm16 = T("idxm16", I16)
            nc.vector.tensor_copy(idxm16[:], idxm[:])

            # scatter payloads
            vu16 = v[:].bitcast(U16).rearrange("p (s two) -> p s two", two=2)
            vlo = T("vlo", U16); vhi = T("vhi", U16)
            nc.vector.tensor_copy(vlo[:], vu16[:, :, 0])
            nc.vector.tensor_copy(vhi[:], vu16[:, :, 1])
            ubf = T("ubf", BF16)
            nc.vector.tensor_copy(ubf[:], u[:])

            # op l2norm (use the output buffer as scratch for the squares --
            # it is rewritten by the final assembly below)
            osb = pb.tile([128, 9 * D], F32, tag="osb", name="osb")
            osq = osb[:, 0:S * 7].rearrange("p (s c) -> p s c", c=7)
            TT(osq, a3[:, :, C_OP:C_OP + 7], a3[:, :, C_OP:C_OP + 7], OP.mult)
            ssq = T("ssq")
            nc.vector.tensor_reduce(ssq[:], osq, mybir.AxisListType.X, OP.add)
            nrm = T("nrm")
            nc.scalar.activation(nrm[:], ssq[:], AF.Sqrt)
            TS(nrm[:], nrm[:], EPS, None, OP.max)
            inv = T("inv")
            nc.vector.reciprocal(inv[:], nrm[:])
            opn = [T(f"opn{j}", BF16) for j in range(7)]
            for j in range(7):
                TT(opn[j][:], col(C_OP + j), inv[:], OP.mult)

            # scatters
            svlo = T("svlo", U16); svhi = T("svhi", U16)
            su = T("su", BF16)
            nc.gpsimd.local_scatter(svlo[:], vlo[:], idxe16[:], 128, D, S)
            nc.gpsimd.local_scatter(svhi[:], vhi[:], idxe16[:], 128, D, S)
            nc.gpsimd.local_scatter(su[:], ubf[:], idxe16[:], 128, D, S)
            sop = [T(f"sop{j}", BF16) for j in range(7)]
            for j in range(7):
                nc.gpsimd.local_scatter(sop[j][:], opn[j][:], idxm16[:], 128, D, S)

            # assemble output
            dv16 = osb[:, 0:D].bitcast(U16).rearrange("p (s two) -> p s two", two=2)
            nc.vector.tensor_copy(dv16[:, :, 0], svlo[:])
            nc.vector.tensor_copy(dv16[:, :, 1], svhi[:])
            nc.vector.tensor_copy(osb[:, D:2 * D], su[:])
            opout = osb[:, 2 * D:9 * D].rearrange("p (d c) -> p d c", c=7)
            for j in range(7):
                nc.vector.tensor_copy(opout[:, :, j], sop[j][:])
            nc.sync.dma_start(out_d[:], osb[:])

            if DEBUG:
                dbg_arrays = [m_, dsg, int_, dec_, dig, fsc, fpm, v, u, c,
                              idxe, idxm, nrm, alpha, beta, w_]
                for k, t in enumerate(dbg_arrays[:16]):
                    nc.sync.dma_start(dbg_d[:, k * S:(k + 1) * S], t[:])

    nc.compile()
    return nc


def kernel(**inputs) -> np.ndarray:
    x = np.asarray(inputs["x"], np.float32)
    start_pos = int(np.asarray(inputs["start_pos"]))
    assert x.shape == (B, S, D)

    key = start_pos
    if "graph" not in _graph_cache:
        _graph_cache["graph"] = _build_graph()
    nc = _graph_cache["graph"]
    wa, wb = _pack_weights(inputs, start_pos)

    in_maps = []
    for i in range(N_CORES):
        in_maps.append({
            "x": x[i * BL:(i + 1) * BL],
            "wa": wa,
            "wb": wb,
        })

    trace = os.environ.get("ARTH_KERNEL_TRACE", "0") == "1"
    res = run_bass_kernel_spmd(nc, in_maps, core_ids=list(range(N_CORES)),
                               trace=trace)
    _graph_cache["last_results"] = res
    out = np.concatenate([res.results[i]["out"] for i in range(N_CORES)], axis=0)
    return out


# revision 6
# speedup vs baseline: 492.5377x; 492.5377x over previous
"""Trainium2 Bass kernel for nn_ArthTextToDenseBlock (8-core data parallel).

Algorithm (mathematically exact reformulation of the reference scan):
  The per-step MLP gate decisions depend only on the token (and the binary
  carry fds for p_denseop, handled by evaluating both branches). So:

  Phase A (batched over all B*S tokens): six MLPs -> packed 32 head values
    per token (argmax decision *differences*, densepred logits, op logits),
    token-major [b, s, 32], staged to DRAM.
  Phase B (per-core, [128b x 512s] arrays): decisions via compares; the
    sequential carries (pointer cumsum, fds sticky bit, fpm decimal scale,
    run value v, run written-flag u) are affine/max recurrences evaluated
    with the hardware tensor_tensor_scan; final placement of per-run values
    into output positions is a per-partition local_scatter (fp32 as two
    16-bit half scatters).

  Output row = [trans_dense(512) | trans_valid(512) | trans_op(512*7)].

Precision modes (ARTH_PRECISION):
  fp32  - true fp32 matmuls (4 PE passes); logit error ~1e-7.
  split - bf16 hi/lo 3-pass split (hi@Whi + hi@Wlo + lo@Whi) for the five
          decision MLPs (~1e-5 logit error); single-pass bf16 for the p_op
          value head (op values only scale the output op columns, whose
          share of the output norm is ~2e-5).
"""
import sys
import os
import numpy as np

try:
    import concourse.bass as bass
except ImportError:
    sys.path.insert(0, "/opt/trn_rl_repo")
    import concourse.bass as bass
import concourse.tile as tile
from concourse import mybir, bacc, library_config
from concourse.bass_utils import run_bass_kernel_spmd

AF = mybir.ActivationFunctionType
OP = mybir.AluOpType
F32 = mybir.dt.float32
BF16 = mybir.dt.bfloat16
U16 = mybir.dt.uint16
I16 = mybir.dt.int16

N_CORES = 8
B, S, D = 1024, 512, 512
BL = B // N_CORES          # 128 batch rows per core
DH2 = 256
EPS = 1e-5
N_STILES = int(os.environ.get("ARTH_NTILES", S // 4))  # tiles of 4 s-slices

MLPS = ["p_valid", "p_move", "p_op", "p_dec", "p_densepred", "p_denseop"]
OPI = 2                    # index of p_op in MLPS
NHEADV = 7                 # valid, move, op, dec, densepred, den0, den1

# packed head columns
C_DV = 0
C_DM20, C_DM21 = 1, 2
C_DD = 3
C_DEN0 = 4
C_DEN1 = 9
C_DP = 14
C_OP = 24
NHC = 32

DEBUG = os.environ.get("ARTH_KERNEL_DEBUG", "0") == "1"
REPEAT = int(os.environ.get("ARTH_REPEAT", "1"))
PRECISION = os.environ.get("ARTH_PRECISION", "split")
assert PRECISION in ("fp32", "split")

_graph_cache = {}


class Off:
    """Column offsets into the fp32 blob [128, WA] (and bf16 blob in split)."""
    def __init__(self, split):
        self.split = split
        if split:
            self.b1 = 0
        else:
            self.w1 = 0
            self.w2 = self.w1 + 6 * 16 * 128
            self.w3 = self.w2 + 6 * 8 * 128
            self.b1 = self.w3 + 14 * 32
        self.b2 = self.b1 + 28
        self.b3 = self.b2 + 12
        self.idn = self.b3 + 32
        self.ones = self.idn + 128
        self.WA = self.ones + 512
        # bf16 blob (split mode): hi/lo interleaved per chunk
        self.f_w1 = 0
        self.f_w2 = self.f_w1 + 6 * 16 * 2 * 128
        self.f_w3 = self.f_w2 + 6 * 8 * 2 * 128
        self.WBF = self.f_w3 + 14 * 2 * 32

    def w1_off(self, m, kc, mc):
        return self.w1 + ((m * 4 + kc) * 4 + mc) * 128

    def w2_off(self, m, kc, mc2):
        return self.w2 + ((m * 4 + kc) * 2 + mc2) * 128

    def w3_off(self, j):
        return self.w3 + j * 32

    def f_w1_off(self, m, kc, mc, h):
        return self.f_w1 + (((m * 4 + kc) * 4 + mc) * 2 + h) * 128

    def f_w2_off(self, m, kc, mc2, h):
        return self.f_w2 + (((m * 4 + kc) * 2 + mc2) * 2 + h) * 128

    def f_w3_off(self, j, h):
        return self.f_w3 + (j * 2 + h) * 32

    def b1_off(self, m, mc):   # m in 0..6 (6 = denseop fds=1 variant)
        return self.b1 + m * 4 + mc

    def b2_off(self, m, mc2):
        return self.b2 + m * 2 + mc2


OFF = Off(PRECISION == "split")
WB_MASK = 0
WB_ONES = 512
WB_W = 1024


def _split_bf16(w):
    import ml_dtypes
    hi = w.astype(ml_dtypes.bfloat16)
    lo = (w - hi.astype(np.float32)).astype(ml_dtypes.bfloat16)
    return hi, lo


def _pack_weights(inputs, start_pos):
    o = OFF
    split = o.split
    wa = np.zeros((128, o.WA), np.float32)
    import ml_dtypes
    wbf = np.zeros((128, o.WBF), ml_dtypes.bfloat16) if split else None

    def arr(t):
        return np.asarray(t, np.float32)

    w3cat = np.zeros((7 * DH2, NHC), np.float32)
    b3cat = np.zeros(NHC, np.float32)

    for m, name in enumerate(MLPS):
        w1, b1, w2, b2, w3, b3 = [arr(t) for t in inputs[name]]
        if name == "p_denseop":
            w1a, w1b = w1[:D], w1[D:]
            svec = w1b.astype(np.float64).sum(0).astype(np.float32)
            w1 = w1a
            b1_eff = [b1, (b1.astype(np.float64) + svec).astype(np.float32)]
        else:
            b1_eff = [b1]

        if split:
            w1h, w1l = _split_bf16(w1)
            w2h, w2l = _split_bf16(w2)
            for kc in range(4):
                for mc in range(4):
                    for h, wsrc in ((0, w1h), (1, w1l)):
                        c0 = o.f_w1_off(m, kc, mc, h)
                        wbf[:, c0:c0 + 128] = \
                            wsrc[kc * 128:(kc + 1) * 128, mc * 128:(mc + 1) * 128]
                for mc2 in range(2):
                    for h, wsrc in ((0, w2h), (1, w2l)):
                        c0 = o.f_w2_off(m, kc, mc2, h)
                        wbf[:, c0:c0 + 128] = \
                            wsrc[kc * 128:(kc + 1) * 128, mc2 * 128:(mc2 + 1) * 128]
        else:
            for kc in range(4):
                for mc in range(4):
                    wa[:, o.w1_off(m, kc, mc):o.w1_off(m, kc, mc) + 128] = \
                        w1[kc * 128:(kc + 1) * 128, mc * 128:(mc + 1) * 128]
                for mc2 in range(2):
                    wa[:, o.w2_off(m, kc, mc2):o.w2_off(m, kc, mc2) + 128] = \
                        w2[kc * 128:(kc + 1) * 128, mc2 * 128:(mc2 + 1) * 128]

        for mc in range(4):
            wa[:, o.b1_off(m, mc)] = b1_eff[0][mc * 128:(mc + 1) * 128]
        if name == "p_denseop":
            for mc in range(4):
                wa[:, o.b1_off(6, mc)] = b1_eff[1][mc * 128:(mc + 1) * 128]
        for mc2 in range(2):
            wa[:, o.b2_off(m, mc2)] = b2[mc2 * 128:(mc2 + 1) * 128]

        def blk(v):
            return slice(v * DH2, (v + 1) * DH2)
        w3d = w3.astype(np.float64)
        b3d = b3.astype(np.float64)
        if name == "p_valid":
            w3cat[blk(0), C_DV] = w3d[:, 1] - w3d[:, 0]
            b3cat[C_DV] = b3d[1] - b3d[0]
        elif name == "p_move":
            w3cat[blk(1), C_DM20] = w3d[:, 2] - w3d[:, 0]
            w3cat[blk(1), C_DM21] = w3d[:, 2] - w3d[:, 1]
            b3cat[C_DM20] = b3d[2] - b3d[0]
            b3cat[C_DM21] = b3d[2] - b3d[1]
        elif name == "p_op":
            w3cat[blk(2), C_OP:C_OP + 7] = w3
            b3cat[C_OP:C_OP + 7] = b3
        elif name == "p_dec":
            w3cat[blk(3), C_DD] = w3d[:, 1] - w3d[:, 0]
            b3cat[C_DD] = b3d[1] - b3d[0]
        elif name == "p_densepred":
            w3cat[blk(4), C_DP:C_DP + 10] = w3
            b3cat[C_DP:C_DP + 10] = b3
        elif name == "p_denseop":
            for v, base in ((5, C_DEN0), (6, C_DEN1)):
                w3cat[blk(v), base + 0] = w3d[:, 1] - w3d[:, 0]
                w3cat[blk(v), base + 1] = w3d[:, 1] - w3d[:, 2]
                w3cat[blk(v), base + 2] = w3d[:, 1] - w3d[:, 3]
                w3cat[blk(v), base + 3] = w3d[:, 2] - w3d[:, 0]
                w3cat[blk(v), base + 4] = w3d[:, 2] - w3d[:, 3]
                b3cat[base + 0] = b3d[1] - b3d[0]
                b3cat[base + 1] = b3d[1] - b3d[2]
                b3cat[base + 2] = b3d[1] - b3d[3]
                b3cat[base + 3] = b3d[2] - b3d[0]
                b3cat[base + 4] = b3d[2] - b3d[3]

    if split:
        w3h, w3l = _split_bf16(w3cat)
        for j in range(14):
            for h, wsrc in ((0, w3h), (1, w3l)):
                wbf[:, o.f_w3_off(j, h):o.f_w3_off(j, h) + NHC] = \
                    wsrc[j * 128:(j + 1) * 128, :]
    else:
        for j in range(14):
            wa[:, o.w3_off(j):o.w3_off(j) + NHC] = w3cat[j * 128:(j + 1) * 128, :]
    wa[0, o.b3:o.b3 + NHC] = b3cat
    wa[:, o.idn:o.idn + 128] = np.eye(128, dtype=np.float32)
    wa[:, o.ones:o.ones + 512] = 1.0

    wb = np.zeros((128, WB_W), np.float32)
    wb[:, WB_MASK + start_pos:WB_MASK + S] = 1.0
    wb[:, WB_ONES:WB_ONES + S] = 1.0
    return wa, wbf, wb


def _build_graph():
    o = OFF
    split = o.split
    nc = bacc.Bacc("TRN2", target_bir_lowering=False, debug=False,
                   num_devices=N_CORES)
    x_d = nc.dram_tensor("x", (BL, S, D), F32, kind="ExternalInput")
    wa_d = nc.dram_tensor("wa", (128, o.WA), F32, kind="ExternalInput")
    if split:
        wbf_d = nc.dram_tensor("wbf", (128, o.WBF), BF16, kind="ExternalInput")
    wb_d = nc.dram_tensor("wb", (128, WB_W), F32, kind="ExternalInput")
    acc_d = nc.dram_tensor("acc", (128, S * NHC), F32, kind="ExternalOutput")
    out_d = nc.dram_tensor("out", (BL, 9 * D), F32, kind="ExternalOutput")
    if DEBUG:
        dbg_d = nc.dram_tensor("dbg", (128, 16 * S), F32, kind="ExternalOutput")

    for _rep in range(REPEAT):
        # ---------------- Phase A ----------------
        with tile.TileContext(nc) as tc:
            nc.gpsimd.load_library(library_config.local_scatter)
            with tc.tile_pool(name="const", bufs=1) as cpool, \
                 tc.tile_pool(name="xin", bufs=2) as xinp, \
                 tc.tile_pool(name="xT", bufs=2) as xTp, \
                 tc.tile_pool(name="h1", bufs=1) as h1p_, \
                 tc.tile_pool(name="h1b", bufs=1) as h1bp, \
                 tc.tile_pool(name="h2", bufs=2) as h2p_, \
                 tc.tile_pool(name="hsb", bufs=2) as hsbp, \
                 tc.tile_pool(name="stg", bufs=2) as stgp, \
                 tc.tile_pool(name="ps1", bufs=2, space="PSUM") as ps1, \
                 tc.tile_pool(name="ps2", bufs=2, space="PSUM") as ps2, \
                 tc.tile_pool(name="psh", bufs=2, space="PSUM") as psh, \
                 tc.tile_pool(name="pst", bufs=2, space="PSUM") as pst:

                wa = cpool.tile([128, o.WA], F32, name="wa")
                nc.sync.dma_start(wa[:], wa_d[:])
                if split:
                    wbf = cpool.tile([128, o.WBF], BF16, name="wbf")
                    nc.sync.dma_start(wbf[:], wbf_d[:])
                idn = wa[:, o.idn:o.idn + 128]

                for i in range(N_STILES):
                    s0 = 4 * i
                    xin = xinp.tile([128, 4 * D], F32, tag="xin", name="xin")
                    nc.sync.dma_start(
                        xin[:], x_d[:, s0:s0 + 4, :].rearrange("p a b -> p (a b)"))

                    if split:
                        xhi = xinp.tile([128, 4 * D], BF16, tag="xhi", name="xhi")
                        xlo = xinp.tile([128, 4 * D], BF16, tag="xlo", name="xlo")
                        nc.vector.tensor_copy(xhi[:], xin[:])
                        nc.vector.tensor_tensor(xlo[:], xin[:], xhi[:], OP.subtract)
                        xThi = xTp.tile([128, 16, 128], BF16, tag="xThi", name="xThi")
                        xTlo = xTp.tile([128, 16, 128], BF16, tag="xTlo", name="xTlo")
                        nc.sync.dma_start_transpose(xThi[:], xhi[:])
                        nc.sync.dma_start_transpose(xTlo[:], xlo[:])
                        xThiv = xThi[:].rearrange("p (so kc) b -> p kc so b", kc=4)
                        xTlov = xTlo[:].rearrange("p (so kc) b -> p kc so b", kc=4)

                        def x_rhs(sel, kc):
                            return (xThiv if sel == 0 else xTlov)[:, kc]
                    else:
                        xT = xTp.tile([128, 4, 512], F32, tag="xT", name="xT")
                        for so in range(4):
                            for kc in range(4):
                                trp = pst.tile([128, 128], F32, tag="tr", name="tr")
                                nc.tensor.transpose(
                                    trp[:],
                                    xin[:, so * D + kc * 128: so * D + (kc + 1) * 128],
                                    idn)
                                nc.vector.tensor_copy(
                                    xT[:, kc, so * 128:(so + 1) * 128], trp[:])

                    hp = psh.tile([32, 512], F32, tag="heads", name="heads")
                    nc.tensor.matmul(hp[:], wa[0:1, o.b3:o.b3 + NHC],
                                     wa[0:1, o.ones:o.ones + 512],
                                     start=True, stop=False, skip_group_check=True)

                    hv = 0
                    for m, name in enumerate(MLPS):
                        is_den = name == "p_denseop"
                        is_op = m == OPI
                        if split:
                            # (x source: 0=hi 1=lo, weight half: 0=hi 1=lo)
                            passes = [(0, 0)] if is_op else [(0, 0), (0, 1), (1, 0)]
                            h1hi = h1p_.tile([128, 4, 512], BF16, tag="h1hi", name="h1hi")
                            h1lo = None if is_op else \
                                h1p_.tile([128, 4, 512], BF16, tag="h1lo", name="h1lo")
                            hb = [(h1hi, h1lo)]
                            if is_den:
                                h1bhi = h1bp.tile([128, 4, 512], BF16, tag="h1bhi", name="h1bhi")
                                h1blo = h1bp.tile([128, 4, 512], BF16, tag="h1blo", name="h1blo")
                                hb.append((h1bhi, h1blo))
                            for mc in range(4):
                                p1 = ps1.tile([128, 512], F32, tag="ps1", name="ps1")
                                np_ = len(passes)
                                for pi, (xs, h) in enumerate(passes):
                                    for kc in range(4):
                                        nc.tensor.matmul(
                                            p1[:],
                                            wbf[:, o.f_w1_off(m, kc, mc, h):
                                                o.f_w1_off(m, kc, mc, h) + 128],
                                            x_rhs(xs, kc),
                                            start=(pi == 0 and kc == 0),
                                            stop=(pi == np_ - 1 and kc == 3))
                                for vi, (thi, tlo) in enumerate(hb):
                                    bia = wa[:, o.b1_off(6 if vi else m, mc):
                                             o.b1_off(6 if vi else m, mc) + 1]
                                    nc.scalar.activation(thi[:, mc, :], p1[:],
                                                         AF.Lrelu, bias=bia,
                                                         scale=1.0, alpha=0.01)
                                    if tlo is not None:
                                        h1f = h2p_.tile([128, 512], F32,
                                                        tag="h1f", name="h1f")
                                        nc.scalar.activation(h1f[:], p1[:],
                                                             AF.Lrelu, bias=bia,
                                                             scale=1.0, alpha=0.01)
                                        nc.vector.tensor_tensor(
                                            tlo[:, mc, :], h1f[:], thi[:, mc, :],
                                            OP.subtract)
                            for thi, tlo in hb:
                                h2hi = h2p_.tile([128, 2, 512], BF16, tag="h2hi", name="h2hi")
                                h2lo = None if is_op else \
                                    h2p_.tile([128, 2, 512], BF16, tag="h2lo", name="h2lo")
                                for mc2 in range(2):
                                    p2 = ps2.tile([128, 512], F32, tag="ps2", name="ps2")
                                    src = [(thi, 0)] if is_op else \
                                        [(thi, 0), (thi, 1), (tlo, 0)]
                                    np2 = len(src)
                                    for pi, (hsrc, h) in enumerate(src):
                                        for kc in range(4):
                                            nc.tensor.matmul(
                                                p2[:],
                                                wbf[:, o.f_w2_off(m, kc, mc2, h):
                                                    o.f_w2_off(m, kc, mc2, h) + 128],
                                                hsrc[:, kc, :],
                                                start=(pi == 0 and kc == 0),
                                                stop=(pi == np2 - 1 and kc == 3))
                                    bia = wa[:, o.b2_off(m, mc2):o.b2_off(m, mc2) + 1]
                                    nc.scalar.activation(h2hi[:, mc2, :], p2[:],
                                                         AF.Lrelu, bias=bia,
                                                         scale=1.0, alpha=0.01)
                                    if h2lo is not None:
                                        h2f = h2p_.tile([128, 512], F32,
                                                        tag="h1f", name="h2f")
                                        nc.scalar.activation(h2f[:], p2[:],
                                                             AF.Lrelu, bias=bia,
                                                             scale=1.0, alpha=0.01)
                                        nc.vector.tensor_tensor(
                                            h2lo[:, mc2, :], h2f[:], h2hi[:, mc2, :],
                                            OP.subtract)
                                hsrc3 = [(h2hi, 0)] if is_op else \
                                    [(h2hi, 0), (h2hi, 1), (h2lo, 0)]
                                for kc3 in range(2):
                                    j = hv * 2 + kc3
                                    for pi, (hsrc, h) in enumerate(hsrc3):
                                        last = (hv == NHEADV - 1) and (kc3 == 1) \
                                            and (pi == len(hsrc3) - 1)
                                        nc.tensor.matmul(
                                            hp[:],
                                            wbf[:, o.f_w3_off(j, h):o.f_w3_off(j, h) + NHC],
                                            hsrc[:, kc3, :], start=False, stop=last,
                                            skip_group_check=True)
                                hv += 1
                        else:
                            h1 = h1p_.tile([128, 4, 512], F32, tag="h1", name="h1")
                            h1b = h1bp.tile([128, 4, 512], F32, tag="h1b", name="h1b") if is_den else None
                            for mc in range(4):
                                p1 = ps1.tile([128, 512], F32, tag="ps1", name="ps1")
                                for kc in range(4):
                                    nc.tensor.matmul(
                                        p1[:], wa[:, o.w1_off(m, kc, mc):o.w1_off(m, kc, mc) + 128],
                                        xT[:, kc, :], start=(kc == 0), stop=(kc == 3))
                                nc.scalar.activation(
                                    h1[:, mc, :], p1[:], AF.Lrelu,
                                    bias=wa[:, o.b1_off(m, mc):o.b1_off(m, mc) + 1],
                                    scale=1.0, alpha=0.01)
                                if is_den:
                                    nc.scalar.activation(
                                        h1b[:, mc, :], p1[:], AF.Lrelu,
                                        bias=wa[:, o.b1_off(6, mc):o.b1_off(6, mc) + 1],
                                        scale=1.0, alpha=0.01)
                            for h1var in ([h1] if not is_den else [h1, h1b]):
                                h2 = h2p_.tile([128, 2, 512], F32, tag="h2", name="h2")
                                for mc2 in range(2):
                                    p2 = ps2.tile([128, 512], F32, tag="ps2", name="ps2")
                                    for kc in range(4):
                                        nc.tensor.matmul(
                                            p2[:], wa[:, o.w2_off(m, kc, mc2):o.w2_off(m, kc, mc2) + 128],
                                            h1var[:, kc, :], start=(kc == 0), stop=(kc == 3))
                                    nc.scalar.activation(
                                        h2[:, mc2, :], p2[:], AF.Lrelu,
                                        bias=wa[:, o.b2_off(m, mc2):o.b2_off(m, mc2) + 1],
                                        scale=1.0, alpha=0.01)
                                for kc3 in range(2):
                                    j = hv * 2 + kc3
                                    last = (hv == NHEADV - 1) and (kc3 == 1)
                                    nc.tensor.matmul(
                                        hp[:], wa[:, o.w3_off(j):o.w3_off(j) + NHC],
                                        h2[:, kc3, :], start=False, stop=last,
                                        skip_group_check=True)
                                hv += 1

                    hs = hsbp.tile([32, 512], F32, tag="hs", name="hs")
                    nc.vector.tensor_copy(hs[:], hp[:])
                    stg = stgp.tile([128, 4 * NHC], F32, tag="stg", name="stg")
                    for so in range(4):
                        htp = pst.tile([128, 128], F32, tag="tr", name="tr")
                        nc.tensor.transpose(
                            htp[:, :NHC], hs[:, so * 128:(so + 1) * 128],
                            idn[0:32, 0:32])
                        nc.vector.tensor_copy(
                            stg[:, so * NHC:(so + 1) * NHC], htp[:, :NHC])
                    nc.sync.dma_start(acc_d[:, s0 * NHC:(s0 + 4) * NHC], stg[:])

        # ---------------- Phase B ----------------
        with tile.TileContext(nc) as tc:
            with tc.tile_pool(name="pb", bufs=1) as pb:
                acc = pb.tile([128, S * NHC], F32, tag="acc", name="acc")
                nc.sync.dma_start(acc[:], acc_d[:])
                wbt = pb.tile([128, WB_W], F32, tag="wb", name="wbt")
                nc.sync.dma_start(wbt[:], wb_d[:])
                mask = wbt[:, WB_MASK:WB_MASK + S]
                ones = wbt[:, WB_ONES:WB_ONES + S]

                a3 = acc[:].rearrange("p (s c) -> p s c", c=NHC)

                def col(j):
                    return a3[:, :, j]

                def T(tag, dtype=F32, n=S):
                    return pb.tile([128, n], dtype, tag=tag, name=tag)

                TT, TS, STT = nc.vector.tensor_tensor, nc.vector.tensor_scalar, \
                    nc.vector.scalar_tensor_tensor

                nig = T("nig"); TS(nig[:], col(C_DV), 0.0, None, OP.is_le)
                t20 = T("t20"); TS(t20[:], col(C_DM20), 0.0, None, OP.is_gt)
                t21 = T("t21"); TS(t21[:], col(C_DM21), 0.0, None, OP.is_gt)
                mv2 = T("mv2"); TT(mv2[:], t20[:], t21[:], OP.logical_and)
                m_ = T("m_")
                TT(m_[:], nig[:], mv2[:], OP.logical_and)
                TT(m_[:], m_[:], mask, OP.mult)
                dsg = T("dsg")
                TS(dsg[:], col(C_DD), 0.0, None, OP.is_gt)
                TT(dsg[:], dsg[:], mask, OP.mult)

                dens = {}
                for tagb, base in (("d0", C_DEN0), ("d1", C_DEN1)):
                    a1 = T("da1"); TS(a1[:], col(base + 0), 0.0, None, OP.is_gt)
                    a2 = T("da2"); TS(a2[:], col(base + 1), 0.0, None, OP.is_ge)
                    a3_ = T("da3"); TS(a3_[:], col(base + 2), 0.0, None, OP.is_ge)
                    i1 = T("i1" + tagb)
                    TT(i1[:], a1[:], a2[:], OP.logical_and)
                    TT(i1[:], i1[:], a3_[:], OP.logical_and)
                    b1c = T("da1"); TS(b1c[:], col(base + 3), 0.0, None, OP.is_gt)
                    b2c = T("da2"); TS(b2c[:], col(base + 1), 0.0, None, OP.is_lt)
                    b3c = T("da3"); TS(b3c[:], col(base + 4), 0.0, None, OP.is_ge)
                    i2 = T("i2" + tagb)
                    TT(i2[:], b1c[:], b2c[:], OP.logical_and)
                    TT(i2[:], i2[:], b3c[:], OP.logical_and)
                    dens[tagb] = (i1, i2)

                mx = T("mx")
                nc.vector.tensor_reduce(mx[:], a3[:, :, C_DP:C_DP + 10],
                                        mybir.AxisListType.X, OP.max)
                dig = T("dig"); dig2 = T("dig2")
                nc.vector.memset(dig[:], 0.0)
                cur, nxt = dig, dig2
                cj = T("cj")
                for j in range(1, 10):
                    TT(cj[:], col(C_DP + j), mx[:], OP.is_ge)
                    STT(nxt[:], cj[:], float(j), cur[:], OP.mult, OP.add)
                    cur, nxt = nxt, cur
                dig = cur

                nm = T("nm"); TS(nm[:], m_[:], -1.0, 1.0, OP.mult, OP.add)
                d1f = T("d1f"); TT(d1f[:], nm[:], dsg[:], OP.mult)
                fsc = T("fsc")
                nc.vector.tensor_tensor_scan(fsc[:], nm[:], d1f[:], 0.0, OP.mult, OP.max)
                fp = T("fp")
                nc.vector.memset(fp[:, 0:1], 0.0)
                nc.vector.tensor_copy(fp[:, 1:S], fsc[:, 0:S - 1])

                def sel(i0, i1, tag):
                    d = T("seld"); TT(d[:], i1[:], i0[:], OP.subtract)
                    q = T(tag)
                    TT(q[:], fp[:], d[:], OP.mult)
                    TT(q[:], q[:], i0[:], OP.add)
                    TT(q[:], q[:], nig[:], OP.mult)
                    TT(q[:], q[:], mask, OP.mult)
                    return q
                int_ = sel(dens["d0"][0], dens["d1"][0], "int_")
                dec_ = sel(dens["d0"][1], dens["d1"][1], "dec_")

                nd = T("nd"); TS(nd[:], dec_[:], -1.0, 1.0, OP.mult, OP.add)
                t3 = T("t3"); TT(t3[:], nd[:], nm[:], OP.mult)
                af = T("af"); STT(af[:], dec_[:], 0.1, t3[:], OP.mult, OP.add)
                fpm = T("fpm")
                nc.vector.tensor_tensor_scan(fpm[:], af[:], m_[:], 1.0, OP.mult, OP.add)
                fpmp = T("fpmp")
                nc.vector.memset(fpmp[:, 0:1], 1.0)
                nc.vector.tensor_copy(fpmp[:, 1:S], fpm[:, 0:S - 1])

                alpha = T("alpha")
                STT(alpha[:], int_[:], 9.0, ones, OP.mult, OP.add)
                TT(alpha[:], alpha[:], nm[:], OP.mult)
                b1v = T("b1v")
                TT(b1v[:], dig[:], fpmp[:], OP.mult)
                TS(b1v[:], b1v[:], 0.1, None, OP.mult)
                TT(b1v[:], b1v[:], dec_[:], OP.mult)
                beta = T("beta")
                TT(beta[:], int_[:], dig[:], OP.mult)
                TT(beta[:], beta[:], b1v[:], OP.add)
                v = T("v")
                nc.vector.tensor_tensor_scan(v[:], alpha[:], beta[:], 0.0, OP.mult, OP.add)

                w_ = T("w_"); TT(w_[:], int_[:], dec_[:], OP.add)
                u = T("u")
                nc.vector.tensor_tensor_scan(u[:], nm[:], w_[:], 0.0, OP.mult, OP.max)

                c = T("c")
                nc.vector.tensor_tensor_scan(c[:], ones, m_[:], 0.0, OP.mult, OP.add)

                e = T("e")
                nc.vector.memset(e[:, S - 1:S], 1.0)
                nc.vector.tensor_copy(e[:, 0:S - 1], m_[:, 1:S])

                ok = T("ok"); TS(ok[:], c[:], float(D) - 0.5, None, OP.is_le)
                cp1 = T("cp1"); TS(cp1[:], c[:], 1.0, None, OP.add)
                mske = T("mske"); TT(mske[:], e[:], ok[:], OP.mult)
                idxe = T("idxe")
                TT(idxe[:], cp1[:], mske[:], OP.mult)
                TS(idxe[:], idxe[:], 1.0, None, OP.subtract)
                idxe16 = T("idxe16", I16)
                nc.vector.tensor_copy(idxe16[:], idxe[:])
                mskm = T("mskm"); TT(mskm[:], m_[:], ok[:], OP.mult)
                idxm = T("idxm")
                TT(idxm[:], cp1[:], mskm[:], OP.mult)
                TS(idxm[:], idxm[:], 1.0, None, OP.subtract)
                idxm16 = T("idxm16", I16)
                nc.vector.tensor_copy(idxm16[:], idxm[:])

                vu16 = v[:].bitcast(U16).rearrange("p (s two) -> p s two", two=2)
                vlo = T("vlo", U16); vhi = T("vhi", U16)
                nc.vector.tensor_copy(vlo[:], vu16[:, :, 0])
                nc.vector.tensor_copy(vhi[:], vu16[:, :, 1])
                ubf = T("ubf", BF16)
                nc.vector.tensor_copy(ubf[:], u[:])

                osb = pb.tile([128, 9 * D], F32, tag="osb", name="osb")
                osq = osb[:, 0:S * 7].rearrange("p (s c) -> p s c", c=7)
                TT(osq, a3[:, :, C_OP:C_OP + 7], a3[:, :, C_OP:C_OP + 7], OP.mult)
                ssq = T("ssq")
                nc.vector.tensor_reduce(ssq[:], osq, mybir.AxisListType.X, OP.add)
                nrm = T("nrm")
                nc.scalar.activation(nrm[:], ssq[:], AF.Sqrt)
                TS(nrm[:], nrm[:], EPS, None, OP.max)
                inv = T("inv")
                nc.vector.reciprocal(inv[:], nrm[:])
                opn = [T(f"opn{j}", BF16) for j in range(7)]
                for j in range(7):
                    TT(opn[j][:], col(C_OP + j), inv[:], OP.mult)

                svlo = T("svlo", U16); svhi = T("svhi", U16)
                su = T("su", BF16)
                nc.gpsimd.local_scatter(svlo[:], vlo[:], idxe16[:], 128, D, S)
                nc.gpsimd.local_scatter(svhi[:], vhi[:], idxe16[:], 128, D, S)
                nc.gpsimd.local_scatter(su[:], ubf[:], idxe16[:], 128, D, S)
                sop = [T(f"sop{j}", BF16) for j in range(7)]
                for j in range(7):
                    nc.gpsimd.local_scatter(sop[j][:], opn[j][:], idxm16[:], 128, D, S)

                dv16 = osb[:, 0:D].bitcast(U16).rearrange("p (s two) -> p s two", two=2)
                nc.vector.tensor_copy(dv16[:, :, 0], svlo[:])
                nc.vector.tensor_copy(dv16[:, :, 1], svhi[:])
                nc.vector.tensor_copy(osb[:, D:2 * D], su[:])
                opout = osb[:, 2 * D:9 * D].rearrange("p (d c) -> p d c", c=7)
                for j in range(7):
                    nc.vector.tensor_copy(opout[:, :, j], sop[j][:])
                nc.sync.dma_start(out_d[:], osb[:])

                if DEBUG:
                    dbg_arrays = [m_, dsg, int_, dec_, dig, fsc, fpm, v, u, c,
                                  idxe, idxm, nrm, alpha, beta, w_]
                    for k, t in enumerate(dbg_arrays[:16]):
                        nc.sync.dma_start(dbg_d[:, k * S:(k + 1) * S], t[:])

    nc.compile()
    return nc


def kernel(**inputs) -> np.ndarray:
    x = np.asarray(inputs["x"], np.float32)
    start_pos = int(np.asarray(inputs["start_pos"]))
    assert x.shape == (B, S, D)

    if "graph" not in _graph_cache:
        _graph_cache["graph"] = _build_graph()
    nc = _graph_cache["graph"]
    wa, wbf, wb = _pack_weights(inputs, start_pos)

    in_maps = []
    for i in range(N_CORES):
        m = {"x": x[i * BL:(i + 1) * BL], "wa": wa, "wb": wb}
        if wbf is not None:
            m["wbf"] = wbf
        in_maps.append(m)

    trace = os.environ.get("ARTH_KERNEL_TRACE", "0") == "1"
    res = run_bass_kernel_spmd(nc, in_maps, core_ids=list(range(N_CORES)),
                               trace=trace)
    _graph_cache["last_results"] = res
    out = np.concatenate([res.results[i]["out"] for i in range(N_CORES)], axis=0)
    return out
